# revision 27
# baseline (speedup 1.0000x reference)
"""AdapLSNet MLP kernel for 8 TRN2 NeuronCores (data-parallel).

reference:
    h  = elu(x @ W0 + b0)
    h  = elu(h @ W1 + b1)
    out = sigmoid(h @ W2 + b2)          # [B, 1]
    alpha = piecewise(out)               # a=0.1, b=0.2, c=0.8
    returns (out, alpha)

Strategy
- Shard batch (32768) across 8 cores (4096 rows each); replicate weights.
- Host pre-transposes each x shard to x^T [1024, 4096] so every layer's
  activations live in [feature(partitions), batch(free)] layout; weights
  W0/W1 are already in [K, M] layout for the stationary operand.  No
  on-device transposes.
- Layer 1 in float32r (full-rate fp32 PE path, ~1.5e-4 matmul rel err);
  layers 2+3 in fp16 (same PE rate, FWL weight loads, half the bytes).
- Single fused pass: W0 (f32r, 64KB/part) and W1 (fp16, 64KB/part) are
  both SBUF-resident, so h1 never leaves the chip.  Weights are stored
  as per-strip tiles so DMA completion is tracked at strip granularity
  and compute starts as soon as the first strips land.
- Software pipeline: L1 runs one batch-chunk ahead of L2
  (L1_0, L1_1, L2_0, L1_2, L2_1, ... L2_7) so the PE has L1 work while
  W1 streams in at startup.
- elu(z) = min(exp(z) - 1, relu(z)): 2 ScalarE LUT ops reading PSUM with
  the bias fused + 1 fused VectorE (e-1) min r op.
- alpha = relu(-0.5*out + 0.1) + relu(0.5*out - 0.4)  (exact identity for
  out in [0,1]).
"""

import numpy as np

BATCH = 32768
DIN = 1024
DH = 2048
NCORES = 8
SHARD = BATCH // NCORES          # 4096
CHUNK = 512
NCH = SHARD // CHUNK             # 8
KI = DIN // 128                  # 8
KH = DH // 128                   # 16
MH = DH // 128                   # 16
NH1S = 32                        # h1 slots (fp16 [128,512], 1KB each)
NXTS = 14                        # xt slots (f32r [128,512], 2KB each)
W0S = 8                          # W0 strips of [128, 256] per k-slab
W1S = 4                          # W1 strips of [128, 512] per k-slab


def _install_profile_shim():
    """Allow trace=True under axon (exec_time_ns capture) if possible."""
    import sys
    import types

    try:
        import antenv

        if "antenv.axon_hooks" in sys.modules:
            return
        mod = types.ModuleType("antenv.axon_hooks")
        _hook = [None]
        mod.set_axon_ntff_profile_hook = lambda h: _hook.__setitem__(0, h)
        mod.get_axon_ntff_profile_hook = lambda: _hook[0]
        sys.modules["antenv.axon_hooks"] = mod
        antenv.axon_hooks = mod
        try:
            from trn_agent_boot.trn_boot import _ntff_profile_via_ctypes

            mod.set_axon_ntff_profile_hook(
                _ntff_profile_via_ctypes("/opt/axon/libaxon_pjrt.so")
            )
        except Exception:
            pass
    except Exception:
        pass


_NC_CACHE = None


def _build():
    global _NC_CACHE
    if _NC_CACHE is not None:
        return _NC_CACHE

    import concourse.mybir as mybir
    import concourse.tile as tile
    from concourse import bacc

    F32R = mybir.dt.float32r
    F32 = mybir.dt.float32
    F16 = mybir.dt.float16
    AF = mybir.ActivationFunctionType
    ALU = mybir.AluOpType

    nc = bacc.Bacc("TRN2", target_bir_lowering=False)

    xt_ext = nc.declare_dram_parameter("xt", [DIN, SHARD], F32R, isOutput=False)
    w0_ext = nc.declare_dram_parameter("w0", [DIN, DH], F32R, isOutput=False)
    w1_ext = nc.declare_dram_parameter("w1", [DH, DH], F16, isOutput=False)
    w2_ext = nc.declare_dram_parameter("w2", [128, KH], F16, isOutput=False)
    b0_ext = nc.declare_dram_parameter("b0", [128, MH], F32, isOutput=False)
    b1_ext = nc.declare_dram_parameter("b1", [128, MH], F32, isOutput=False)
    b2_ext = nc.declare_dram_parameter("b2", [1, 1], F32, isOutput=False)
    out_ext = nc.declare_dram_parameter("out", [1, SHARD], F32, isOutput=True)
    alpha_ext = nc.declare_dram_parameter("alpha", [1, SHARD], F32, isOutput=True)

    with tile.TileContext(nc) as tc:
        with (
            tc.tile_pool(name="w0p", bufs=1) as w0p,
            tc.tile_pool(name="w1p", bufs=1) as w1p,
            tc.tile_pool(name="xtp", bufs=1) as xtp,
            tc.tile_pool(name="h1p", bufs=1) as h1p,
            tc.tile_pool(name="hpool", bufs=2) as hpool,
            tc.tile_pool(name="h2p", bufs=5) as h2p,
            tc.tile_pool(name="redp", bufs=3) as redp,
            tc.tile_pool(name="cst", bufs=1) as cst,
            tc.tile_pool(name="ps", bufs=6, space="PSUM") as ps,
            tc.tile_pool(name="ops", bufs=2, space="PSUM") as ops,
        ):
            # weights live in full [128, DH] slabs: LDWEIGHTS from small
            # tiles measures 1.2-2.5x slower, so slabs stay big and DMA
            # completion is slab-granular
            w0_sb = [
                w0p.tile([128, DH], F32R, tag=f"w0_{k}", name=f"w0_{k}")
                for k in range(KI)
            ]
            w1_sb = [
                w1p.tile([128, DH], F16, tag=f"w1_{k}", name=f"w1_{k}")
                for k in range(KH)
            ]

            def w0_lhsT(k, m):
                return w0_sb[k][:, m * 128:(m + 1) * 128]

            def w1_lhsT(k, m):
                return w1_sb[k][:, m * 128:(m + 1) * 128]

            def emit_xt(n, halves=False):
                tiles = []
                base = (KI * n) % NXTS
                for k in range(KI):
                    t = xtp.tile(
                        [128, CHUNK], F32R, tag=f"xt{(base + k) % NXTS}",
                        name=f"xt_{n}_{k}",
                    )
                    src = xt_ext[k * 128:(k + 1) * 128,
                                 n * CHUNK:(n + 1) * CHUNK]
                    if halves:
                        nc.sync.dma_start(t[:, 0:256], src[:, 0:256])
                        nc.sync.dma_start(t[:, 256:512], src[:, 256:512])
                    else:
                        nc.sync.dma_start(t[:], src)
                    tiles.append(t)
                return tiles

            # weight slab DMAs: 4 strips per slab, alternating the sync
            # (HWDGE) and gpsimd (SWDGE) queue families to double the
            # prefetch bandwidth; slab-major so early slabs finish first
            def emit_w(sb_tiles, ext, nk, elem, nstrip, ks, mid=None):
                for k in ks:
                    for s in range(nstrip):
                        eng = nc.sync if (s % 2 == 0) else nc.gpsimd
                        eng.dma_start(
                            sb_tiles[k][:, s * elem:(s + 1) * elem],
                            ext[k * 128:(k + 1) * 128, s * elem:(s + 1) * elem],
                        )
                    if mid is not None and k == mid[0]:
                        mid[1]()

            # startup-ordered DMA stream (first-use first).  xt1 is
            # injected between W0 slabs 2 and 3 so L1(1) is never starved
            # (SWDGE/gpsimd carries only latency-tolerant weight strips).
            xt_tiles = {0: emit_xt(0, halves=True)}
            emit_w(
                w0_sb, w0_ext, KI, DH // 8, 8, range(KI),
                mid=(2, lambda: xt_tiles.__setitem__(1, emit_xt(1, halves=True))),
            )
            emit_w(w1_sb, w1_ext, KH, DH // 4, 4, range(KH))

            w2_sb = cst.tile([128, KH], F16, tag="w2", name="w2")
            nc.sync.dma_start(w2_sb[:], w2_ext[:])
            b0_sb = cst.tile([128, MH], F32, tag="b0", name="b0")
            nc.sync.dma_start(b0_sb[:], b0_ext[:])
            b1_sb = cst.tile([128, MH], F32, tag="b1", name="b1")
            nc.sync.dma_start(b1_sb[:], b1_ext[:])
            b2_sb = cst.tile([1, 1], F32, tag="b2", name="b2")
            nc.sync.dma_start(b2_sb[:], b2_ext[:])
            c_b1 = cst.tile([1, 1], F32, tag="c_b1", name="c_b1")
            c_b2 = cst.tile([1, 1], F32, tag="c_b2", name="c_b2")
            c_sn = cst.tile([1, 1], F32, tag="c_sn", name="c_sn")
            c_sp = cst.tile([1, 1], F32, tag="c_sp", name="c_sp")
            nc.vector.memset(c_b1[:], 0.1)
            nc.vector.memset(c_b2[:], -0.4)
            nc.vector.memset(c_sn[:], -0.5)
            nc.vector.memset(c_sp[:], 0.5)

            h1_tiles = {}

            def l1_chunk(n):
                """L1: h1(n) = elu(W0.T @ xT(n) + b0), kept in SBUF."""
                xt_sb = xt_tiles.pop(n)
                h1base = (MH * n) % NH1S
                tiles = []
                for m in range(MH):
                    psum = ps.tile([128, CHUNK], F32, tag="ps",
                                   name=f"psA_{n}_{m}")
                    for k in range(KI):
                        nc.tensor.matmul(
                            psum[:], w0_lhsT(k, m), xt_sb[k][:],
                            start=(k == 0), stop=(k == KI - 1),
                        )
                    e = hpool.tile([128, CHUNK], F32, tag="e", name="e")
                    r = hpool.tile([128, CHUNK], F32, tag="r", name="r")
                    nc.scalar.activation(e[:], psum[:], AF.Exp,
                                         bias=b0_sb[:, m:m + 1])
                    nc.scalar.activation(r[:], psum[:], AF.Relu,
                                         bias=b0_sb[:, m:m + 1])
                    h1 = h1p.tile(
                        [128, CHUNK], F16, tag=f"h{(h1base + m) % NH1S}",
                        name=f"h1_{n}_{m}",
                    )
                    nc.vector.scalar_tensor_tensor(
                        h1[:], e[:], 1.0, r[:], ALU.subtract, ALU.min
                    )
                    tiles.append(h1)
                h1_tiles[n] = tiles

            def l2_chunk(n):
                """L2 + L3 + sigmoid + alpha for chunk n.

                L3 (h2 @ W2, M=1) is packed 4-wide into PE column groups
                via tile_position, issued as bursts of 4 concurrent MMs;
                the 4 partial rows (psum partitions 0/32/64/96) are
                reduced on ScalarE/VectorE.
                """
                h1_sb = h1_tiles.pop(n)
                out_ps = ops.tile([128, CHUNK], F32, tag="ops",
                                  name=f"outps_{n}")
                h2_burst = []
                for m in range(MH):
                    psum = ps.tile([128, CHUNK], F32, tag="ps",
                                   name=f"psB_{n}_{m}")
                    for k in range(KH):
                        nc.tensor.matmul(
                            psum[:], w1_lhsT(k, m), h1_sb[k][:],
                            start=(k == 0), stop=(k == KH - 1),
                        )
                    e = hpool.tile([128, CHUNK], F32, tag="e", name="e")
                    r = hpool.tile([128, CHUNK], F32, tag="r", name="r")
                    nc.scalar.activation(e[:], psum[:], AF.Exp,
                                         bias=b1_sb[:, m:m + 1])
                    nc.scalar.activation(r[:], psum[:], AF.Relu,
                                         bias=b1_sb[:, m:m + 1])
                    h2 = h2p.tile([128, CHUNK], F16, tag="h2", name="h2")
                    nc.vector.scalar_tensor_tensor(
                        h2[:], e[:], 1.0, r[:], ALU.subtract, ALU.min
                    )
                    h2_burst.append((m, h2))
                    if len(h2_burst) == 4:
                        for mm, hh in h2_burst:
                            g = mm % 4
                            nc.tensor.matmul(
                                out_ps[32 * g:32 * g + 1, :],
                                w2_sb[:, mm:mm + 1], hh[:],
                                start=(mm < 4), stop=(mm >= MH - 4),
                                tile_position=(0, 32 * g),
                            )
                        h2_burst = []
                # reduce 4 partial rows -> z, then sigmoid + alpha
                t0 = redp.tile([1, CHUNK], F32, tag="tred", name="t0")
                nc.scalar.activation(t0[:], out_ps[0:1, :], AF.Copy)
                t1 = redp.tile([1, CHUNK], F32, tag="tred", name="t1")
                nc.vector.tensor_tensor(t1[:], t0[:], out_ps[32:33, :], ALU.add)
                t2 = redp.tile([1, CHUNK], F32, tag="tred", name="t2")
                nc.vector.tensor_tensor(t2[:], t1[:], out_ps[64:65, :], ALU.add)
                z = redp.tile([1, CHUNK], F32, tag="tred", name="z")
                nc.vector.tensor_tensor(z[:], t2[:], out_ps[96:97, :], ALU.add)
                o = hpool.tile([1, CHUNK], F32, tag="e", name="o")
                nc.scalar.activation(o[:], z[:], AF.Sigmoid, bias=b2_sb[:])
                r1 = hpool.tile([1, CHUNK], F32, tag="r", name="r1")
                r2 = redp.tile([1, CHUNK], F32, tag="tred", name="r2")
                nc.scalar.activation(r1[:], o[:], AF.Relu,
                                     bias=c_b1[:], scale=c_sn[:])
                nc.scalar.activation(r2[:], o[:], AF.Relu,
                                     bias=c_b2[:], scale=c_sp[:])
                al = hpool.tile([1, CHUNK], F32, tag="e", name="al")
                nc.vector.tensor_tensor(al[:], r1[:], r2[:], ALU.add)
                nc.sync.dma_start(out_ext[0:1, n * CHUNK:(n + 1) * CHUNK], o[:])
                nc.sync.dma_start(
                    alpha_ext[0:1, n * CHUNK:(n + 1) * CHUNK], al[:]
                )

            # pipeline: L1 one chunk ahead of L2
            l1_chunk(0)
            for n in range(1, NCH):
                l1_chunk(n)
                if n + 1 < NCH:
                    xt_tiles[n + 1] = emit_xt(n + 1)
                l2_chunk(n - 1)
            l2_chunk(NCH - 1)

    nc.compile()
    _NC_CACHE = nc
    return nc


LAST_RESULTS = None


def kernel(x, W0, b0, W1, b1, W2, b2):
    global LAST_RESULTS
    _install_profile_shim()
    from concourse.bass_utils import run_bass_kernel_spmd

    x = np.asarray(x, dtype=np.float32)
    W0 = np.ascontiguousarray(np.asarray(W0, dtype=np.float32))
    W1 = np.ascontiguousarray(np.asarray(W1, dtype=np.float32))
    W2 = np.asarray(W2, dtype=np.float32)
    b0 = np.asarray(b0, dtype=np.float32)
    b1 = np.asarray(b1, dtype=np.float32)
    b2 = np.asarray(b2, dtype=np.float32)

    nc = _build()

    w1h = W1.astype(np.float16)
    w2h = np.ascontiguousarray(W2.astype(np.float16).reshape(KH, 128).T)
    b0r = np.ascontiguousarray(b0.reshape(MH, 128).T)
    b1r = np.ascontiguousarray(b1.reshape(MH, 128).T)
    b2r = b2.reshape(1, 1)

    in_maps = []
    for c in range(NCORES):
        shard = x[c * SHARD:(c + 1) * SHARD]
        in_maps.append(
            {
                "xt": np.ascontiguousarray(shard.T),
                "w0": W0,
                "w1": w1h,
                "w2": w2h,
                "b0": b0r,
                "b1": b1r,
                "b2": b2r,
            }
        )

    res = run_bass_kernel_spmd(nc, in_maps, core_ids=list(range(NCORES)))
    LAST_RESULTS = res

    out = np.concatenate([res.results[c]["out"][0] for c in range(NCORES)])
    alpha = np.concatenate([res.results[c]["alpha"][0] for c in range(NCORES)])
    return out[:, None].astype(np.float32), alpha[:, None].astype(np.float32)
